# revision 21
# baseline (speedup 1.0000x reference)
"""Multi-head sparse attention TRN2 Bass kernel (v2: fp16 datapath).

Problem: B=2, S=4096, D=512, H=8, HD=64; learned top-k (256/batch) column
sparsity; the union of both batches' top-k key columns (<=512) is shared
across batch/heads.

Strategy:
- Host (cheap, <3% of FLOPs): importance scorer gelu(x@Ws1+bs1)@Ws2+bs2 in
  float64, per-batch top-k, union -> selected column index list (padded to a
  multiple of 128 slots).
- Device (8 cores): core c handles batch b=c//4, query rows qc=c%4 (1024
  rows each), computing all 8 heads. All matmul operands are fp16 (PSUM
  accumulation stays f32), which halves DMA traffic and keeps full PE rate.
    QT[d,q] from xT chunks and Wq; KT[d,slot], V[slot,d] from the gathered
    selected rows xsel.
    per head pair: S^T[slot,q] = KT-slice x QT-slice matmuls (K=64),
    P = exp(scale*S) (scores are O(6), no max-subtraction needed),
    numer^T[64+1,q] = [V_h | maskcol]^T x P via matmuls; the mask column
    gives the softmax denominator (pad slots have V rows exactly zero; no
    V bias on device - bv is folded into bo on the host:
    O = numer/den + bv, so Y = O@Wo + bo == numer/den@Wo + (bo + bv@Wo)).
    Denominator rows are collected by the (otherwise idle) Pool engine into
    recd[8,q]; DVE reciprocal + an indicator matmul broadcast normalizes.
    Y[q,:] = Oall @ Wo, stored fp16; host adds bo' and casts to f32.
"""

import math
import sys

import numpy as np

if "/opt/trn_rl_repo" not in sys.path:
    sys.path.insert(0, "/opt/trn_rl_repo")

B, S, D, H = 2, 4096, 512, 8
HD = D // H  # 64
DK = 256
NCORES = 8
QS = S // 4  # 1024 query rows per core
SCALE = HD ** -0.5

_cache = {}


def _erf(x):
    try:
        from scipy.special import erf
        return erf(x)
    except ImportError:
        return np.vectorize(math.erf)(x)


def _host_topk_union(x, Ws1, bs1, Ws2, bs2, top_k):
    """Importance scores in float64 -> per-batch top-k -> sorted union."""
    x64 = x.astype(np.float64)
    h = x64.reshape(-1, D) @ Ws1.astype(np.float64) + bs1.astype(np.float64)
    g = 0.5 * h * (1.0 + _erf(h / math.sqrt(2.0)))
    imp = (g @ Ws2.astype(np.float64) + bs2.astype(np.float64)).reshape(B, S)
    k = max(1, min(int(top_k), S))
    if k >= S:
        return np.arange(S)
    idx = np.argpartition(-imp, k - 1, axis=1)[:, :k]
    return np.unique(idx)


def _build_program(NS):
    import concourse.bacc as bacc
    import concourse.mybir as mybir
    import concourse.tile as tile

    F32 = mybir.dt.float32
    F16 = mybir.dt.float16
    AF = mybir.ActivationFunctionType
    MUL = mybir.AluOpType.mult

    NK = NS // 128  # selected-slot chunks of 128
    NQ = QS // 512  # 512-wide query chunks (2)

    nc = bacc.Bacc(
        "TRN2",
        target_bir_lowering=False,
        debug=False,
        enable_asserts=False,
        num_devices=NCORES,
    )

    # paired loads: [wk_ki | xsT_ki] per ki so the first KT matmul's inputs
    # arrive in one DMA
    kin_d = nc.dram_tensor("kin", (128, 4 * (D + NS)), F16,
                           kind="ExternalInput")
    qin_d = nc.dram_tensor("qin", (128, 4 * (D + 512)), F16,
                           kind="ExternalInput")
    vin_d = nc.dram_tensor("vin", (128, 4 * D), F16, kind="ExternalInput")
    xq1_d = nc.dram_tensor("xq1", (128, 4 * 512), F16, kind="ExternalInput")
    wout_d = nc.dram_tensor("wout", (128, 4 * D), F16, kind="ExternalInput")
    constf_d = nc.dram_tensor("constf", (128, 8), F32, kind="ExternalInput")
    cons16_d = nc.dram_tensor("cons16", (128, 8 * NK), F16,
                              kind="ExternalInput")
    indic_d = nc.dram_tensor("indic", (1, 128), F16, kind="ExternalInput")
    y_d = nc.dram_tensor("y", (QS, D), F16, kind="ExternalOutput")

    PW = D + NS  # kin pair width
    QW = D + 512  # qin pair width

    with tile.TileContext(nc) as tc:
        with tc.tile_pool(name="big", bufs=1) as bp, \
             tc.tile_pool(name="work", bufs=1) as wp, \
             tc.tile_pool(name="ps", bufs=1, space="PSUM") as pp:
            # ---- SBUF tiles ----
            kin_sb = [bp.tile([128, PW], F16, name=f"kin{i}")
                      for i in range(4)]
            qin_sb = [bp.tile([128, QW], F16, name=f"qin{i}")
                      for i in range(4)]
            wv2_sb = [bp.tile([128, 2 * D], F16, name=f"wv2_{i}")
                      for i in range(2)]
            xq12_sb = [bp.tile([128, 1024], F16, name=f"xq12_{i}")
                       for i in range(2)]
            wo2_sb = [bp.tile([128, 2 * D], F16, name=f"wo2_{i}")
                      for i in range(2)]
            constf_sb = bp.tile([128, 8], F32)
            mcol8_sb = bp.tile([128, 8 * NK], F16)
            indic_sb = bp.tile([1, 128], F16)

            # ---- PE p-state warm-up: the tensor engine ramps to full clock
            # only after ~3us of sustained work, so burn the ramp on dummy
            # matmuls over a zeroed tile while the first DMAs are in flight
            zt = bp.tile([128, 512], F16, name="warmzt")
            nc.vector.memset(zt[:], 0.0)
            for wi in range(5):
                pw = pp.tile([128, 512], F32, tag="score", bufs=2)
                nc.tensor.matmul(pw[:], zt[:, 0:128], zt[:],
                                 start=True, stop=True)

            # ---- loads (ordered by first use) ----
            for i in range(4):
                nc.sync.dma_start(kin_sb[i][:],
                                  kin_d.ap()[:, i * PW:(i + 1) * PW])
            nc.sync.dma_start(constf_sb[:], constf_d.ap())
            for i in range(4):
                nc.sync.dma_start(qin_sb[i][:],
                                  qin_d.ap()[:, i * QW:(i + 1) * QW])
            for i in range(2):
                nc.sync.dma_start(wv2_sb[i][:],
                                  vin_d.ap()[:, i * 2 * D:(i + 1) * 2 * D])
            nc.sync.dma_start(mcol8_sb[:], cons16_d.ap())
            nc.sync.dma_start(indic_sb[:], indic_d.ap())
            for i in range(2):
                nc.sync.dma_start(xq12_sb[i][:],
                                  xq1_d.ap()[:, i * 1024:(i + 1) * 1024])
            for i in range(2):
                nc.sync.dma_start(wo2_sb[i][:],
                                  wout_d.ap()[:, i * 2 * D:(i + 1) * 2 * D])

            def wk(i):
                return kin_sb[i][:, 0:D]

            def xsT(i):
                return kin_sb[i][:, D:PW]

            def wq(i):
                return qin_sb[i][:, 0:D]

            def xqT(i, j):
                if j == 0:
                    return qin_sb[i][:, D:QW]
                return xq12_sb[i // 2][:, (i % 2) * 512:(i % 2 + 1) * 512]

            def wv_c(i):
                return wv2_sb[i // 2][:, (i % 2) * D:(i % 2 + 1) * D]

            def wo_c(i):
                return wo2_sb[i // 2][:, (i % 2) * D:(i % 2 + 1) * D]

            bqc = constf_sb[:, 0:4]
            bkc = constf_sb[:, 4:8]

            # ---- projections ----
            kt_sb = [bp.tile([128, NS], F16, name=f"kt{mi}")
                     for mi in range(4)]
            qt_sb = [bp.tile([128, QS], F16, name=f"qt{mi}")
                     for mi in range(4)]

            def kt_block(mi):
                pk = pp.tile([128, NS], F32, tag="projbc", bufs=2)
                for ki in range(4):
                    nc.tensor.matmul(
                        pk[:], wk(ki)[:, mi * 128:(mi + 1) * 128], xsT(ki),
                        start=(ki == 0), stop=(ki == 3))
                # ACT is idle before the first exps; use it for early evacs
                nc.scalar.activation(kt_sb[mi][:], pk[:], AF.Identity,
                                     bias=bkc[:, mi:mi + 1], scale=1.0)

            def qt_block(mi, nj):
                pq = pp.tile([128, 512], F32, tag="projbc", bufs=2)
                for ki in range(4):
                    nc.tensor.matmul(
                        pq[:], wq(ki)[:, mi * 128:(mi + 1) * 128],
                        xqT(ki, nj),
                        start=(ki == 0), stop=(ki == 3))
                if nj == 0:
                    nc.scalar.activation(
                        qt_sb[mi][:, 0:512], pq[:], AF.Identity,
                        bias=bqc[:, mi:mi + 1], scale=1.0)
                else:
                    nc.vector.tensor_scalar_add(
                        qt_sb[mi][:, nj * 512:(nj + 1) * 512], pq[:],
                        bqc[:, mi:mi + 1])

            def qt_pass(nj):
                for mi in range(4):
                    qt_block(mi, nj)

            vaug_sb = []

            def v_block(si):
                pv = pp.tile([128, D], F32, tag="ot", bufs=2)
                for ki in range(4):
                    nc.tensor.matmul(
                        pv[:], xsT(ki)[:, si * 128:(si + 1) * 128], wv_c(ki),
                        start=(ki == 0), stop=(ki == 3))
                t = bp.tile([128, 8 * 65], F16, name=f"vaug{si}")
                v3 = t[:, 0:520].rearrange("p (h c) -> p h c", c=65)
                nc.vector.tensor_copy(
                    v3[:, :, 0:64],
                    pv[:, 0:512].rearrange("p (h c) -> p h c", c=64))
                nc.vector.tensor_copy(
                    v3[:, :, 64:65].rearrange("p h c -> p (h c)"),
                    mcol8_sb[:, si * 8:(si + 1) * 8])
                vaug_sb.append(t)

            for mi in range(4):
                kt_block(mi)
                if mi < NK:
                    v_block(mi)
            qt_pass(0)

            # ---- attention, with per-pair normalize so the tail only waits
            # on the last head pair ----
            oall_sb = [bp.tile([128, QS], F16, name=f"oall{t}")
                       for t in range(4)]
            pyp = {}  # pre-accumulated out-proj psums for the last 2 chunks

            def store_y(qc, ysb, cols, engq):
                engq.dma_start(y_d.ap()[qc * 128:(qc + 1) * 128, cols],
                               ysb[:, cols])

            for qj in range(NQ):
                qs = slice(qj * 512, (qj + 1) * 512)
                for t in range(4):
                    exps = {}
                    for si in range(NK):
                        # both heads of the pair share one [128,1024] psum
                        # tile / one Exp op
                        with tc.high_priority():
                            psc = pp.tile([128, 1024], F32, tag="score",
                                          bufs=2)
                            for hh in range(2):
                                po = hh * 64
                                nc.tensor.matmul(
                                    psc[:, hh * 512:(hh + 1) * 512],
                                    kt_sb[t][po:po + 64,
                                             si * 128:(si + 1) * 128],
                                    qt_sb[t][po:po + 64, qs],
                                    start=True, stop=True)
                            ex = wp.tile([128, 1024], F16, tag="exp", bufs=6)
                            nc.scalar.activation(ex[:], psc[:], AF.Exp,
                                                 scale=SCALE)
                        exps[si] = ex
                    recru = wp.tile([1, 1024], F16, tag="recr", bufs=4,
                                    name=f"recr{qj}_{t}")
                    for hh in range(2):
                        h = 2 * t + hh
                        pot = pp.tile([65, 512], F32, tag="ot", bufs=2)
                        for si in range(NK):
                            nc.tensor.matmul(
                                pot[:],
                                vaug_sb[si][:, h * 65:h * 65 + 65],
                                exps[si][:, hh * 512:(hh + 1) * 512],
                                start=(si == 0), stop=(si == NK - 1))
                        # softmax denominator (mask row 64) -> reciprocal
                        # (both heads' rows live on partition 0: engine APs
                        # may only start at partition 0/32/64/96)
                        with nc.allow_low_precision(
                                reason="fp16 softmax denom"):
                            nc.vector.reciprocal(
                                recru[0:1, hh * 512:(hh + 1) * 512],
                                pot[64:65, :])
                        # unnormalized numerators -> oall rows (Pool/GPSIMD
                        # cannot read PSUM, so these stay on DVE)
                        nc.vector.tensor_copy(
                            oall_sb[t][hh * 64:hh * 64 + 64, qs],
                            pot[0:64, :])
                    # broadcast 1/den over the pair's 128 rows (shares the
                    # pot psum rotation) and normalize in one TT
                    pbc = pp.tile([128, 512], F32, tag="ot", bufs=2)
                    nc.tensor.matmul(pbc[0:64, :], indic_sb[0:1, 0:64],
                                     recru[0:1, 0:512],
                                     start=True, stop=True)
                    nc.tensor.matmul(pbc[64:128, :], indic_sb[0:1, 64:128],
                                     recru[0:1, 512:1024],
                                     start=True, stop=True,
                                     skip_group_check=True)
                    nc.vector.tensor_tensor(oall_sb[t][:, qs],
                                            oall_sb[t][:, qs], pbc[:], MUL)
                    if qj == NQ - 1:
                        # accumulate the last two output chunks pair-by-pair
                        # so only one matmul layer remains after the last TT
                        for pi, qc in enumerate((6, 7)):
                            if t == 0:
                                pyp[pi] = pp.tile([128, D], F32,
                                                  tag="projbc", bufs=2,
                                                  name=f"pyp{pi}")
                            nc.tensor.matmul(
                                pyp[pi][:],
                                oall_sb[t][:, qc * 128:(qc + 1) * 128],
                                wo_c(t), start=(t == 0), stop=(t == 3))

                # queue the next q-half's QT to fill attention gaps
                if qj + 1 < NQ:
                    qt_pass(qj + 1)

                # ---- Y[q, :] = Oall @ Wo for this q range ----
                # row-pair stores: two 128-row chunks per DMA halves the
                # HWDGE issue serialization at the tail
                def pair_store(qc0, ysb2, engq):
                    dst = y_d.ap()[qc0 * 128:(qc0 + 2) * 128, :]
                    engq.dma_start(
                        dst.rearrange("(two p) c -> p two c", two=2),
                        ysb2[:].rearrange("p (two c) -> p two c", two=2))

                if qj == 0:
                    for qc in range(4):
                        py = pp.tile([128, D], F32, tag="projbc", bufs=2)
                        for ki in range(4):
                            nc.tensor.matmul(
                                py[:],
                                oall_sb[ki][:, qc * 128:(qc + 1) * 128],
                                wo_c(ki), start=(ki == 0), stop=(ki == 3))
                        if qc % 2 == 0:
                            ysb2 = wp.tile([128, 2 * D], F16, tag="y",
                                           bufs=2, name=f"ysb{qj}_{qc}")
                            nc.vector.tensor_copy(ysb2[:, 0:D], py[:])
                        else:
                            nc.vector.tensor_copy(ysb2[:, D:2 * D], py[:])
                            pair_store(qc - 1, ysb2, nc.sync)
                else:
                    for qc in (4, 5):
                        py = pp.tile([128, D], F32, tag="score", bufs=2)
                        for ki in range(4):
                            nc.tensor.matmul(
                                py[:],
                                oall_sb[ki][:, qc * 128:(qc + 1) * 128],
                                wo_c(ki), start=(ki == 0), stop=(ki == 3))
                        if qc == 4:
                            ysb2 = wp.tile([128, 2 * D], F16, tag="y",
                                           bufs=2, name=f"ysb{qj}_45")
                            nc.vector.tensor_copy(ysb2[:, 0:D], py[:])
                        else:
                            nc.vector.tensor_copy(ysb2[:, D:2 * D], py[:])
                            pair_store(4, ysb2, nc.sync)
                    # chunks 6,7 are already accumulated; only evac + store,
                    # split across engines/queues for the shortest tail
                    ysb3 = wp.tile([128, 2 * D], F16, tag="y", bufs=2,
                                   name="ysb67")
                    nc.scalar.copy(ysb3[:, 0:D], pyp[0][:])
                    pair_store_cols = y_d.ap()[6 * 128:7 * 128, :]
                    nc.scalar.dma_start(pair_store_cols, ysb3[:, 0:D])
                    nc.vector.tensor_copy(ysb3[:, D:D + 256],
                                          pyp[1][:, 0:256])
                    nc.sync.dma_start(
                        y_d.ap()[7 * 128:8 * 128, 0:256],
                        ysb3[:, D:D + 256])
                    nc.scalar.copy(ysb3[:, D + 256:2 * D],
                                   pyp[1][:, 256:512])
                    nc.scalar.dma_start(
                        y_d.ap()[7 * 128:8 * 128, 256:512],
                        ysb3[:, D + 256:2 * D])

    nc.compile()
    return nc


def _get_program(NS):
    if NS not in _cache:
        _cache[NS] = _build_program(NS)
    return _cache[NS]


def _chunks(arrT, width):
    """(512, W) transposed input -> list of 4 (128, W) chunks."""
    return [np.ascontiguousarray(arrT[i * 128:(i + 1) * 128])
            for i in range(4)]


def kernel(x, Wq, bq, Wk, bk, Wv, bv, Wo, bo, Ws1, bs1, Ws2, bs2, top_k):
    from concourse import bass_utils

    x = np.ascontiguousarray(np.asarray(x, dtype=np.float32))
    Wq = np.asarray(Wq, np.float32)
    bq = np.asarray(bq, np.float32)
    Wk = np.asarray(Wk, np.float32)
    bk = np.asarray(bk, np.float32)
    Wv = np.asarray(Wv, np.float32)
    bv = np.asarray(bv, np.float32)
    Wo = np.asarray(Wo, np.float32)
    bo = np.asarray(bo, np.float32)

    uniq = _host_topk_union(x, np.asarray(Ws1, np.float32),
                            np.asarray(bs1, np.float32),
                            np.asarray(Ws2, np.float32),
                            np.asarray(bs2, np.float32), top_k)
    U = len(uniq)
    NS = max(128, ((U + 127) // 128) * 128)
    NK = NS // 128

    mask = np.zeros(NS, np.float32)
    mask[:U] = 1.0

    constf = np.zeros((128, 8), np.float32)
    constf[:, 0:4] = bq.reshape(4, 128).T
    constf[:, 4:8] = bk.reshape(4, 128).T

    indic = np.ones((1, 128), np.float16)
    mcol8 = np.zeros((128, 8 * NK), np.float16)
    for si in range(NK):
        mcol8[:, si * 8:(si + 1) * 8] = mask[si * 128:(si + 1) * 128, None]

    wqc = _chunks(Wq.astype(np.float16), D)
    wkc = _chunks(Wk.astype(np.float16), D)
    wvc = _chunks(Wv.astype(np.float16), D)
    woc = _chunks(Wo.astype(np.float16), D)
    vin = np.concatenate(wvc, axis=1)
    wout = np.concatenate(woc, axis=1)

    # bo' = bo + bv @ Wo (bv applied after softmax-normalize commutes
    # through the output projection)
    bo_eff = (bo.astype(np.float64)
              + bv.astype(np.float64) @ Wo.astype(np.float64)).astype(
                  np.float32)

    in_maps = []
    for c in range(NCORES):
        b, qcq = divmod(c, 4)
        xq = x[b, qcq * QS:(qcq + 1) * QS, :]          # (1024, 512)
        xqTc = _chunks(np.ascontiguousarray(xq.T).astype(np.float16), QS)
        xs = np.zeros((NS, D), np.float32)
        xs[:U] = x[b, uniq, :]
        xsTc = _chunks(np.ascontiguousarray(xs.T).astype(np.float16), NS)
        kin = np.concatenate(
            [np.concatenate([wkc[i], xsTc[i]], axis=1) for i in range(4)],
            axis=1)
        qin = np.concatenate(
            [np.concatenate([wqc[i], xqTc[i][:, 0:512]], axis=1)
             for i in range(4)], axis=1)
        xq1 = np.concatenate([xqTc[i][:, 512:1024] for i in range(4)],
                             axis=1)
        in_maps.append({
            "kin": kin, "qin": qin, "vin": vin, "xq1": xq1, "wout": wout,
            "constf": constf, "cons16": mcol8, "indic": indic,
        })

    nc = _get_program(NS)
    res = bass_utils.run_bass_kernel_spmd(nc, in_maps,
                                          core_ids=list(range(NCORES)))
    if res.exec_time_ns is not None:
        print(f"HW exec time: {res.exec_time_ns} ns")

    out = np.empty((B, S, D), np.float32)
    for c in range(NCORES):
        b, qcq = divmod(c, 4)
        out[b, qcq * QS:(qcq + 1) * QS, :] = res.results[c]["y"].astype(
            np.float32)
    out += bo_eff[None, None, :]
    return out


# revision 29
# speedup vs baseline: 1.0192x; 1.0192x over previous
"""Multi-head sparse attention TRN2 Bass kernel (v2: fp16 datapath).

Problem: B=2, S=4096, D=512, H=8, HD=64; learned top-k (256/batch) column
sparsity; the union of both batches' top-k key columns (<=512) is shared
across batch/heads.

Strategy:
- Host (cheap, <3% of FLOPs): importance scorer gelu(x@Ws1+bs1)@Ws2+bs2 in
  float64, per-batch top-k, union -> selected column index list (padded to a
  multiple of 128 slots).
- Device (8 cores): core c handles batch b=c//4, query rows qc=c%4 (1024
  rows each), computing all 8 heads. All matmul operands are fp16 (PSUM
  accumulation stays f32), which halves DMA traffic and keeps full PE rate.
    QT[d,q] from xT chunks and Wq; KT[d,slot], V[slot,d] from the gathered
    selected rows xsel.
    per head pair: S^T[slot,q] = KT-slice x QT-slice matmuls (K=64),
    P = exp(scale*S) (scores are O(6), no max-subtraction needed),
    numer^T[64+1,q] = [V_h | maskcol]^T x P via matmuls; the mask column
    gives the softmax denominator (pad slots have V rows exactly zero; no
    V bias on device - bv is folded into bo on the host:
    O = numer/den + bv, so Y = O@Wo + bo == numer/den@Wo + (bo + bv@Wo)).
    Denominator rows are collected by the (otherwise idle) Pool engine into
    recd[8,q]; DVE reciprocal + an indicator matmul broadcast normalizes.
    Y[q,:] = Oall @ Wo, stored fp16; host adds bo' and casts to f32.
"""

import math
import sys

import numpy as np

if "/opt/trn_rl_repo" not in sys.path:
    sys.path.insert(0, "/opt/trn_rl_repo")

B, S, D, H = 2, 4096, 512, 8
HD = D // H  # 64
DK = 256
NCORES = 8
QS = S // 4  # 1024 query rows per core
SCALE = HD ** -0.5

_cache = {}


def _erf(x):
    try:
        from scipy.special import erf
        return erf(x)
    except ImportError:
        return np.vectorize(math.erf)(x)


def _host_topk_union(x, Ws1, bs1, Ws2, bs2, top_k):
    """Importance scores in float64 -> per-batch top-k -> sorted union."""
    x64 = x.astype(np.float64)
    h = x64.reshape(-1, D) @ Ws1.astype(np.float64) + bs1.astype(np.float64)
    g = 0.5 * h * (1.0 + _erf(h / math.sqrt(2.0)))
    imp = (g @ Ws2.astype(np.float64) + bs2.astype(np.float64)).reshape(B, S)
    k = max(1, min(int(top_k), S))
    if k >= S:
        return np.arange(S)
    idx = np.argpartition(-imp, k - 1, axis=1)[:, :k]
    return np.unique(idx)


def _build_program(NS):
    import concourse.bacc as bacc
    import concourse.mybir as mybir
    import concourse.tile as tile

    F32 = mybir.dt.float32
    F16 = mybir.dt.float16
    AF = mybir.ActivationFunctionType
    MUL = mybir.AluOpType.mult

    NK = NS // 128  # selected-slot chunks of 128
    NQ = QS // 512  # 512-wide query chunks (2)

    nc = bacc.Bacc(
        "TRN2",
        target_bir_lowering=False,
        debug=False,
        enable_asserts=False,
        num_devices=NCORES,
    )

    # paired loads: [wk_ki | xsT_ki] per ki so the first KT matmul's inputs
    # arrive in one DMA
    kin_d = nc.dram_tensor("kin", (128, 4 * (D + NS)), F16,
                           kind="ExternalInput")
    qin_d = nc.dram_tensor("qin", (128, 4 * (D + 512)), F16,
                           kind="ExternalInput")
    vin_d = nc.dram_tensor("vin", (128, 4 * D), F16, kind="ExternalInput")
    xq1_d = nc.dram_tensor("xq1", (128, 4 * 512), F16, kind="ExternalInput")
    wout_d = nc.dram_tensor("wout", (128, 4 * D), F16, kind="ExternalInput")
    constf_d = nc.dram_tensor("constf", (128, 8), F32, kind="ExternalInput")
    cons16_d = nc.dram_tensor("cons16", (128, 8 * NK), F16,
                              kind="ExternalInput")
    ident_d = nc.dram_tensor("ident", (128, 128), F16, kind="ExternalInput")
    y_d = nc.dram_tensor("y", (QS, D), F16, kind="ExternalOutput")

    PW = D + NS  # kin pair width
    QW = D + 512  # qin pair width

    with tile.TileContext(nc) as tc:
        with tc.tile_pool(name="big", bufs=1) as bp, \
             tc.tile_pool(name="work", bufs=1) as wp, \
             tc.tile_pool(name="ps", bufs=1, space="PSUM") as pp:
            # ---- SBUF tiles ----
            kin_sb = [bp.tile([128, PW], F16, name=f"kin{i}")
                      for i in range(4)]
            qin_sb = [bp.tile([128, QW], F16, name=f"qin{i}")
                      for i in range(4)]
            wv2_sb = [bp.tile([128, 2 * D], F16, name=f"wv2_{i}")
                      for i in range(2)]
            xq12_sb = [bp.tile([128, 1024], F16, name=f"xq12_{i}")
                       for i in range(2)]
            wo2_sb = [bp.tile([128, 2 * D], F16, name=f"wo2_{i}")
                      for i in range(2)]
            constf_sb = bp.tile([128, 8], F32)
            mcol8_sb = bp.tile([128, 8 * NK], F16)
            ident_sb = bp.tile([128, 128], F16)

            # ---- PE p-state warm-up: the tensor engine ramps to full clock
            # only after ~3us of sustained work, so burn the ramp on dummy
            # matmuls over a zeroed tile while the first DMAs are in flight
            zt = bp.tile([128, 512], F16, name="warmzt")
            nc.vector.memset(zt[:], 0.0)
            for wi in range(5):
                pw = pp.tile([128, 512], F32, tag="score", bufs=2)
                nc.tensor.matmul(pw[:], zt[:, 0:128], zt[:],
                                 start=True, stop=True)

            # ---- loads (ordered by first use) ----
            for i in range(4):
                nc.sync.dma_start(kin_sb[i][:],
                                  kin_d.ap()[:, i * PW:(i + 1) * PW])
            nc.sync.dma_start(constf_sb[:], constf_d.ap())
            for i in range(4):
                nc.sync.dma_start(qin_sb[i][:],
                                  qin_d.ap()[:, i * QW:(i + 1) * QW])
            for i in range(2):
                nc.sync.dma_start(wv2_sb[i][:],
                                  vin_d.ap()[:, i * 2 * D:(i + 1) * 2 * D])
            nc.sync.dma_start(mcol8_sb[:], cons16_d.ap())
            nc.sync.dma_start(ident_sb[:], ident_d.ap())
            for i in range(2):
                nc.sync.dma_start(xq12_sb[i][:],
                                  xq1_d.ap()[:, i * 1024:(i + 1) * 1024])
            for i in range(2):
                nc.sync.dma_start(wo2_sb[i][:],
                                  wout_d.ap()[:, i * 2 * D:(i + 1) * 2 * D])

            def wk(i):
                return kin_sb[i][:, 0:D]

            def xsT(i):
                return kin_sb[i][:, D:PW]

            def wq(i):
                return qin_sb[i][:, 0:D]

            def xqT(i, j):
                if j == 0:
                    return qin_sb[i][:, D:QW]
                return xq12_sb[i // 2][:, (i % 2) * 512:(i % 2 + 1) * 512]

            def wv_c(i):
                return wv2_sb[i // 2][:, (i % 2) * D:(i % 2 + 1) * D]

            def wo_c(i):
                return wo2_sb[i // 2][:, (i % 2) * D:(i % 2 + 1) * D]

            bqc = constf_sb[:, 0:4]
            bkc = constf_sb[:, 4:8]

            # ---- projections ----
            kt_sb = [bp.tile([128, NS], F16, name=f"kt{mi}")
                     for mi in range(4)]
            qt_sb = [bp.tile([128, QS], F16, name=f"qt{mi}")
                     for mi in range(4)]

            def kt_block(mi):
                pk = pp.tile([128, NS], F32, tag="projbc", bufs=2)
                for ki in range(4):
                    nc.tensor.matmul(
                        pk[:], wk(ki)[:, mi * 128:(mi + 1) * 128], xsT(ki),
                        start=(ki == 0), stop=(ki == 3))
                # ACT is idle before the first exps; use it for early evacs
                nc.scalar.activation(kt_sb[mi][:], pk[:], AF.Identity,
                                     bias=bkc[:, mi:mi + 1], scale=1.0)

            def qt_block(mi, nj):
                pq = pp.tile([128, 512], F32, tag="projbc", bufs=2)
                for ki in range(4):
                    nc.tensor.matmul(
                        pq[:], wq(ki)[:, mi * 128:(mi + 1) * 128],
                        xqT(ki, nj),
                        start=(ki == 0), stop=(ki == 3))
                if nj == 0:
                    nc.scalar.activation(
                        qt_sb[mi][:, 0:512], pq[:], AF.Identity,
                        bias=bqc[:, mi:mi + 1], scale=1.0)
                else:
                    nc.vector.tensor_scalar_add(
                        qt_sb[mi][:, nj * 512:(nj + 1) * 512], pq[:],
                        bqc[:, mi:mi + 1])

            def qt_pass(nj):
                for mi in range(4):
                    qt_block(mi, nj)

            vaug_sb = []

            def v_block(si):
                pv = pp.tile([128, D], F32, tag="ot", bufs=2)
                for ki in range(4):
                    nc.tensor.matmul(
                        pv[:], xsT(ki)[:, si * 128:(si + 1) * 128], wv_c(ki),
                        start=(ki == 0), stop=(ki == 3))
                t = bp.tile([128, 8 * 65], F16, name=f"vaug{si}")
                v3 = t[:, 0:520].rearrange("p (h c) -> p h c", c=65)
                nc.vector.tensor_copy(
                    v3[:, :, 0:64],
                    pv[:, 0:512].rearrange("p (h c) -> p h c", c=64))
                nc.vector.tensor_copy(
                    v3[:, :, 64:65].rearrange("p h c -> p (h c)"),
                    mcol8_sb[:, si * 8:(si + 1) * 8])
                vaug_sb.append(t)

            for mi in range(4):
                kt_block(mi)
                if mi < NK:
                    v_block(mi)
            qt_pass(0)

            # ---- attention, with per-pair normalize so the tail only waits
            # on the last head pair ----
            oall_sb = [bp.tile([128, QS], F16, name=f"oall{t}")
                       for t in range(4)]
            oT_sb = [bp.tile([128, 512], F16, name=f"oT{i}")
                     for i in range(8)]
            pyp = {}  # pre-accumulated out-proj psums for the last 2 chunks

            def store_y(qc, ysb, cols, engq):
                engq.dma_start(y_d.ap()[qc * 128:(qc + 1) * 128, cols],
                               ysb[:, cols])

            def pair_compute(qj, t, exps):
                """pot^T + normalize + transpose for one head pair.
                Deferred one pair behind the score/exp stream so the PE never
                stalls on the pair's last exp before issuing the next pair's
                scores."""
                for hh in range(2):
                    h = 2 * t + hh
                    # out[q, hd] with q on partitions — full PE utilization
                    # (65 moving cols vs 512) and per-partition softmax
                    # normalize via tensor_scalar. All 4 q-chunks of a head
                    # share one psum tile (4 accumulation groups) to keep
                    # the PE->DVE->PE chain coarse-grained.
                    potT4 = pp.tile([128, 4 * 65], F32, tag="ot", bufs=2,
                                    name="potT4")
                    for qcl in range(4):
                        for si in range(NK):
                            nc.tensor.matmul(
                                potT4[:, qcl * 65:(qcl + 1) * 65],
                                exps[si][:, hh * 512 + qcl * 128:
                                         hh * 512 + (qcl + 1) * 128],
                                vaug_sb[si][:, h * 65:h * 65 + 65],
                                start=(si == 0), stop=(si == NK - 1))
                    rc = wp.tile([128, 4], F32, tag="recr", bufs=8,
                                 name="rc")
                    den4 = potT4[:].rearrange(
                        "p (four c) -> p four c", c=65)[:, :, 64:65]
                    with nc.allow_low_precision(
                            reason="fp16 softmax denom"):
                        nc.vector.reciprocal(
                            rc[:].rearrange("p (four c) -> p four c",
                                            c=1), den4)
                    for qcl in range(4):
                        nc.vector.tensor_scalar_mul(
                            oT_sb[qj * 4 + qcl][:, h * 64:h * 64 + 64],
                            potT4[:, qcl * 65:qcl * 65 + 64],
                            rc[:, qcl:qcl + 1])
                # transpose this pair's 128 output features back to d-major
                # for the output projection
                for qcl in range(4):
                    tpp = pp.tile([128, 128], F16, tag="ot", bufs=2,
                                  name="tpp")
                    nc.tensor.matmul(
                        tpp[:], oT_sb[qj * 4 + qcl][:, t * 128:
                                                    (t + 1) * 128],
                        ident_sb[:], is_transpose=True,
                        start=True, stop=True)
                    nc.vector.tensor_copy(
                        oall_sb[t][:, qj * 512 + qcl * 128:
                                   qj * 512 + (qcl + 1) * 128],
                        tpp[:])
                if qj == NQ - 1:
                    # accumulate the last two output chunks pair-by-pair so
                    # only one matmul layer remains at the tail
                    for pi, qc in enumerate((6, 7)):
                        if t == 0:
                            pyp[pi] = pp.tile([128, D], F32,
                                              tag="projbc", bufs=2,
                                              name=f"pyp{pi}")
                        nc.tensor.matmul(
                            pyp[pi][:],
                            oall_sb[t][:, qc * 128:(qc + 1) * 128],
                            wo_c(t), start=(t == 0), stop=(t == 3))

            # row-pair stores: two 128-row chunks per DMA halves the HWDGE
            # issue serialization at the tail
            def pair_store(qc0, ysb2, engq):
                dst = y_d.ap()[qc0 * 128:(qc0 + 2) * 128, :]
                engq.dma_start(
                    dst.rearrange("(two p) c -> p two c", two=2),
                    ysb2[:].rearrange("p (two c) -> p two c", two=2))

            pending_pair = None
            for qj in range(NQ):
                qs = slice(qj * 512, (qj + 1) * 512)
                for t in range(4):
                    exps = {}
                    for si in range(NK):
                        # both heads of the pair share one [128,1024] psum
                        # tile / one Exp op
                        with tc.high_priority():
                            psc = pp.tile([128, 1024], F32, tag="score",
                                          bufs=2)
                            for hh in range(2):
                                po = hh * 64
                                nc.tensor.matmul(
                                    psc[:, hh * 512:(hh + 1) * 512],
                                    kt_sb[t][po:po + 64,
                                             si * 128:(si + 1) * 128],
                                    qt_sb[t][po:po + 64, qs],
                                    start=True, stop=True)
                            ex = wp.tile([128, 1024], F16, tag="exp",
                                         bufs=10)
                            nc.scalar.activation(ex[:], psc[:], AF.Exp,
                                                 scale=SCALE)
                        exps[si] = ex
                    if pending_pair is not None:
                        pair_compute(*pending_pair)
                    pending_pair = (qj, t, exps)

                # flush the last pair before crossing pool-rotation
                # boundaries (avoids PE-order deadlocks on projbc slots)
                pair_compute(*pending_pair)
                pending_pair = None

                if qj + 1 < NQ:
                    # queue the next q-half's QT to fill attention gaps
                    qt_pass(qj + 1)
                    # out-proj for this q-half; runs during the next half's
                    # attention
                    for qc in range(4):
                        py = pp.tile([128, D], F32, tag="projbc", bufs=2)
                        for ki in range(4):
                            nc.tensor.matmul(
                                py[:],
                                oall_sb[ki][:, qc * 128:(qc + 1) * 128],
                                wo_c(ki), start=(ki == 0), stop=(ki == 3))
                        if qc % 2 == 0:
                            ysb2 = wp.tile([128, 2 * D], F16, tag="y",
                                           bufs=2, name=f"ysb0_{qc}")
                            nc.vector.tensor_copy(ysb2[:, 0:D], py[:])
                        else:
                            nc.vector.tensor_copy(ysb2[:, D:2 * D], py[:])
                            pair_store(qc - 1, ysb2, nc.sync)

            # ---- tail: Y for the second q-half ----
            for qc in (4, 5):
                py = pp.tile([128, D], F32, tag="score", bufs=2)
                for ki in range(4):
                    nc.tensor.matmul(
                        py[:], oall_sb[ki][:, qc * 128:(qc + 1) * 128],
                        wo_c(ki), start=(ki == 0), stop=(ki == 3))
                if qc == 4:
                    ysb2 = wp.tile([128, 2 * D], F16, tag="y",
                                   bufs=2, name="ysb1_45")
                    nc.vector.tensor_copy(ysb2[:, 0:D], py[:])
                else:
                    nc.vector.tensor_copy(ysb2[:, D:2 * D], py[:])
                    pair_store(4, ysb2, nc.sync)
            # chunks 6,7 are already accumulated; only evac + store, split
            # across engines/queues for the shortest tail
            ysb3 = wp.tile([128, 2 * D], F16, tag="y", bufs=2,
                           name="ysb67")
            nc.scalar.copy(ysb3[:, 0:D], pyp[0][:])
            nc.scalar.dma_start(y_d.ap()[6 * 128:7 * 128, :], ysb3[:, 0:D])
            nc.vector.tensor_copy(ysb3[:, D:D + 256], pyp[1][:, 0:256])
            nc.sync.dma_start(y_d.ap()[7 * 128:8 * 128, 0:256],
                              ysb3[:, D:D + 256])
            nc.scalar.copy(ysb3[:, D + 256:2 * D], pyp[1][:, 256:512])
            nc.scalar.dma_start(y_d.ap()[7 * 128:8 * 128, 256:512],
                                ysb3[:, D + 256:2 * D])

    nc.compile()
    return nc


def _get_program(NS):
    if NS not in _cache:
        _cache[NS] = _build_program(NS)
    return _cache[NS]


def _chunks(arrT, width):
    """(512, W) transposed input -> list of 4 (128, W) chunks."""
    return [np.ascontiguousarray(arrT[i * 128:(i + 1) * 128])
            for i in range(4)]


def kernel(x, Wq, bq, Wk, bk, Wv, bv, Wo, bo, Ws1, bs1, Ws2, bs2, top_k):
    from concourse import bass_utils

    x = np.ascontiguousarray(np.asarray(x, dtype=np.float32))
    Wq = np.asarray(Wq, np.float32)
    bq = np.asarray(bq, np.float32)
    Wk = np.asarray(Wk, np.float32)
    bk = np.asarray(bk, np.float32)
    Wv = np.asarray(Wv, np.float32)
    bv = np.asarray(bv, np.float32)
    Wo = np.asarray(Wo, np.float32)
    bo = np.asarray(bo, np.float32)

    uniq = _host_topk_union(x, np.asarray(Ws1, np.float32),
                            np.asarray(bs1, np.float32),
                            np.asarray(Ws2, np.float32),
                            np.asarray(bs2, np.float32), top_k)
    U = len(uniq)
    NS = max(128, ((U + 127) // 128) * 128)
    NK = NS // 128

    mask = np.zeros(NS, np.float32)
    mask[:U] = 1.0

    constf = np.zeros((128, 8), np.float32)
    constf[:, 0:4] = bq.reshape(4, 128).T
    constf[:, 4:8] = bk.reshape(4, 128).T

    ident = np.eye(128, dtype=np.float16)
    mcol8 = np.zeros((128, 8 * NK), np.float16)
    for si in range(NK):
        mcol8[:, si * 8:(si + 1) * 8] = mask[si * 128:(si + 1) * 128, None]

    wqc = _chunks(Wq.astype(np.float16), D)
    wkc = _chunks(Wk.astype(np.float16), D)
    wvc = _chunks(Wv.astype(np.float16), D)
    woc = _chunks(Wo.astype(np.float16), D)
    vin = np.concatenate(wvc, axis=1)
    wout = np.concatenate(woc, axis=1)

    # bo' = bo + bv @ Wo (bv applied after softmax-normalize commutes
    # through the output projection)
    bo_eff = (bo.astype(np.float64)
              + bv.astype(np.float64) @ Wo.astype(np.float64)).astype(
                  np.float32)

    in_maps = []
    for c in range(NCORES):
        b, qcq = divmod(c, 4)
        xq = x[b, qcq * QS:(qcq + 1) * QS, :]          # (1024, 512)
        xqTc = _chunks(np.ascontiguousarray(xq.T).astype(np.float16), QS)
        xs = np.zeros((NS, D), np.float32)
        xs[:U] = x[b, uniq, :]
        xsTc = _chunks(np.ascontiguousarray(xs.T).astype(np.float16), NS)
        kin = np.concatenate(
            [np.concatenate([wkc[i], xsTc[i]], axis=1) for i in range(4)],
            axis=1)
        qin = np.concatenate(
            [np.concatenate([wqc[i], xqTc[i][:, 0:512]], axis=1)
             for i in range(4)], axis=1)
        xq1 = np.concatenate([xqTc[i][:, 512:1024] for i in range(4)],
                             axis=1)
        in_maps.append({
            "kin": kin, "qin": qin, "vin": vin, "xq1": xq1, "wout": wout,
            "constf": constf, "cons16": mcol8, "ident": ident,
        })

    nc = _get_program(NS)
    res = bass_utils.run_bass_kernel_spmd(nc, in_maps,
                                          core_ids=list(range(NCORES)))
    if res.exec_time_ns is not None:
        print(f"HW exec time: {res.exec_time_ns} ns")

    out = np.empty((B, S, D), np.float32)
    for c in range(NCORES):
        b, qcq = divmod(c, 4)
        out[b, qcq * QS:(qcq + 1) * QS, :] = res.results[c]["y"].astype(
            np.float32)
    out += bo_eff[None, None, :]
    return out
